# revision 33
# baseline (speedup 1.0000x reference)
"""ButterflyGatingUnit Trainium2 kernel.

Contract: kernel(**inputs) takes the FULL inputs (x: [8, 96, 128, 128] plus
conv weights / scalars) and returns the FULL output [8, 96, 128, 128] f32.
Internally: data-parallel over batch, one example per NeuronCore (8 cores),
via one SPMD Bass program.
"""
import numpy as np
import ml_dtypes
import concourse.bass as bass
import concourse.tile as tile
import concourse.mybir as mybir
from concourse.masks import make_identity
from concourse.bass_utils import run_bass_kernel_spmd
from contextlib import ExitStack

# ---------------------------------------------------------------------------
# Patch TileContext._drain_and_barrier: the walrus build in this toolchain
# rejects Drain instructions carrying more than one sem wait ("Too many sync
# wait commands").  Split the final global-clock wait set across a chain of
# Drain instructions on SP, one wait each.
from concourse.vector_clock import ScopedClock

MAX_WAITS_PER_DRAIN = 1


def _patched_drain_and_barrier(self, tick_clock, wait_clock):
    nc = self.nc
    drain_inst = nc.sync.drain()
    wait_clock.add_sem_waits(
        drain_inst.ins, ScopedClock({None: tick_clock.global_clock})
    )
    inst = drain_inst.ins
    si = inst.sync_info
    waits = list(si.on_wait) if (si and si.on_wait) else []
    if len(waits) > MAX_WAITS_PER_DRAIN:
        si.on_wait = waits[:MAX_WAITS_PER_DRAIN]
        rest = waits[MAX_WAITS_PER_DRAIN:]
        while rest:
            extra = nc.sync.drain()
            extra.ins.sync_info = mybir.SyncInfo(
                on_wait=rest[:MAX_WAITS_PER_DRAIN], on_update=[]
            )
            rest = rest[MAX_WAITS_PER_DRAIN:]

    nc.all_engine_barrier()
    assert self.sems is not None
    popped = nc._tile_sem_poison_stack.pop()
    assert popped is self._sem_poison
    nc.clear_and_free_semaphores(list(self.sems.allocated().values()))
    nc.all_engine_barrier()


tile.TileContext._drain_and_barrier = _patched_drain_and_barrier
# ---------------------------------------------------------------------------

AF = mybir.ActivationFunctionType
ALU = mybir.AluOpType
F32 = mybir.dt.float32
F32R = mybir.dt.float32r
BF16 = mybir.dt.bfloat16
FP8 = mybir.dt.float8e4

C = 96
KK = 9


MAX_WAITS_PER_INST = 1


def _split_multi_waits(nc):
    """This walrus build encodes at most one sem wait per instruction. Hoist
    extra waits onto NoOp carriers inserted just before, on the same engine."""
    f = nc.m.functions[0]
    for blk in f.blocks:
        insts = blk.instructions
        new = []
        changed = False
        ctr = 0
        for inst in insts:
            si = inst.sync_info
            waits = list(si.on_wait) if (si and si.on_wait) else []
            if len(waits) > MAX_WAITS_PER_INST:
                changed = True
                while len(waits) > MAX_WAITS_PER_INST:
                    chunk = waits[:MAX_WAITS_PER_INST]
                    waits = waits[MAX_WAITS_PER_INST:]
                    nop = mybir.InstNoOp(
                        name=f"{inst.name}-wsplit{ctr}", engine=inst.engine,
                        ins=[], outs=[],
                        sync_info=mybir.SyncInfo(on_wait=chunk, on_update=[]))
                    try:
                        nc.register_instruction(nop, overwrite=True)
                    except Exception:
                        pass
                    new.append(nop)
                    ctr += 1
                si.on_wait = waits
            new.append(inst)
        if changed:
            blk.instructions = new


def sub_ap(t_ap, row0, col0, nrow, ncol, rstep, cstep):
    """Strided 3D view [C, nrow, ncol] of a padded [C, PH, PW] SBUF tile."""
    base = t_ap[:, row0, col0]  # AP with offset at (row0, col0), shape [C]
    pstep = t_ap.ap[0][0]
    PW = t_ap.ap[-1][1]  # count of last dim
    row_elems = t_ap.ap[-2][0]  # step of row dim (elements)
    return bass.AP(
        tensor=base.tensor,
        offset=base.offset,
        ap=[[pstep, C], [row_elems * rstep, nrow], [cstep, ncol]],
    )


def build_kernel(nc, H, W, sc1, sc2, res_coef, nc1v, nc2v, repeat=1):
    HW = H * W
    total = float(C * HW)
    Ho = (H - 3) // 3 + 1
    Wo = (W - 3) // 3 + 1
    L = Ho * Wo
    PH, PW = H + 2, W + 4  # interior origin: row 1, col 2 (4B-aligned bf16)
    NT = H // 4            # 4-row blocks
    assert H % 4 == 0 and W % 4 == 0
    eps = 1e-5

    CH = 16 * W            # phase-A chunk: 16 rows
    NCH = H // 16

    # ---------------- DRAM ----------------
    x_in = nc.dram_tensor("x", [C, H, W], F32, kind="ExternalInput").ap()
    aw1t = nc.dram_tensor("aw1t", [C, KK, C], BF16, kind="ExternalInput").ap()
    aw2t = nc.dram_tensor("aw2t", [C, KK, C], BF16, kind="ExternalInput").ap()
    aw3t = nc.dram_tensor("aw3t", [C, KK, C], BF16, kind="ExternalInput").ap()
    wc2t = nc.dram_tensor("wc2t", [C, KK, C], BF16, kind="ExternalInput").ap()
    wf1 = nc.dram_tensor("wf1", [C, C], F32, kind="ExternalInput").ap()
    wf2 = nc.dram_tensor("wf2", [C, C], BF16, kind="ExternalInput").ap()
    bfull = nc.dram_tensor("bfull", [C, 1], F32, kind="ExternalInput").ap()
    out_d = nc.dram_tensor("out", [C, H, W], F32, kind="ExternalOutput").ap()

    with tile.TileContext(nc) as tc, ExitStack() as ctx:
        dram = ctx.enter_context(tc.tile_pool(name="dram", bufs=1, space="DRAM"))
        val_d = dram.tile([C, PH, PW], BF16)
        cv_d = dram.tile([C, H, W], BF16)
        a_d = dram.tile([C, H, W], BF16)

        big = ctx.enter_context(tc.tile_pool(name="big", bufs=1))
        x1pad = big.tile([C, PH, PW], BF16)
        x2pad = big.tile([C, PH, PW], BF16)
        mask_sb = big.tile([C, H * W], FP8)

        wpool = ctx.enter_context(tc.tile_pool(name="wpool", bufs=1))
        aw1_sb = wpool.tile([C, KK, C], BF16)
        aw2_sb = wpool.tile([C, KK, C], BF16)
        aw3_sb = wpool.tile([C, KK, C], BF16)
        wc2_sb = wpool.tile([C, KK, C], BF16)
        wf1_sb = wpool.tile([C, C], F32)
        wf1s_sb = wpool.tile([C, C], BF16)
        wf2_sb = wpool.tile([C, C], BF16)
        wf2s_sb = wpool.tile([C, C], BF16)
        bfull_sb = wpool.tile([C, 1], F32)
        ident = wpool.tile([128, 128], BF16)
        identf = wpool.tile([128, 128], F32)
        ones_c = wpool.tile([C, 1], F32)
        ones_row = wpool.tile([1, C], F32)
        make_identity(nc, ident)
        make_identity(nc, identf)
        nc.vector.memset(ones_c, 1.0)
        nc.vector.memset(ones_row, 1.0)

        # zero pad borders once (interiors are rewritten every iteration)
        for pad in (x1pad, x2pad):
            nc.vector.memset(pad[:, 0, :], 0.0)
            nc.vector.memset(pad[:, PH - 1, :], 0.0)
            nc.vector.memset(pad[:, :, 0:2], 0.0)
            nc.vector.memset(pad[:, :, W + 2:W + 4], 0.0)
        jnk_bf = wpool.tile([C, 512], BF16)
        nc.vector.memset(jnk_bf, 0.0)
        zrow = wpool.tile([C, PW], BF16)
        nc.vector.memset(zrow, 0.0)
        nc.sync.dma_start(out=val_d[:, 0, :], in_=zrow)
        nc.sync.dma_start(out=val_d[:, PH - 1, :], in_=zrow)

        # stats / scalar pools
        st = ctx.enter_context(tc.tile_pool(name="st", bufs=1))
        bnstats = st.tile([C, NT, 6], F32)
        mv = st.tile([C, 2], F32)
        pm = st.tile([C, 2], F32)
        s1_part = st.tile([C, NCH], F32)
        q1_part = st.tile([C, NCH], F32)
        pos_part = st.tile([C, NCH], F32)
        p3 = st.tile([C, 3], F32)
        sc = st.tile([1, 32], F32)      # scalar scratch, [1,1] slices
        cstv = st.tile([1, 16], F32)    # consts to broadcast
        cst = st.tile([C, 16], F32)     # broadcast result
        m2 = st.tile([1, 2], F32)       # (m, -m)
        mb = st.tile([C, 2], F32)       # broadcast (m, -m)
        astats = st.tile([C, NT, 6], F32)
        cvstats = st.tile([C, NT, 6], F32)
        amv = st.tile([C, 2], F32)
        cvmv = st.tile([C, 2], F32)
        lnp = st.tile([C, 4], F32)
        lns = st.tile([1, 8], F32)
        lnb = st.tile([C, 8], F32)
        corr = st.tile([C, 1], F32)
        delcor = st.tile([C, 1], F32)
        e1I = st.tile([C, C], BF16)     # diag(e1) for PE-side xr
        e2I = st.tile([C, C], BF16)
        gI = st.tile([C, C], FP8)
        attn_sb = st.tile([C, KK, C], F32)
        attnT_sb = st.tile([C, KK, C], BF16)
        mx = st.tile([C, 1], F32)
        negmx = st.tile([C, 1], F32)
        den = st.tile([C, 1], F32)
        rden = st.tile([C, 1], F32)
        kct = st.tile([1, 8], F32)   # compile-time consts as [1,1] APs
        nc.vector.memset(kct[:, 0:1], total)
        nc.vector.memset(kct[:, 1:2], eps)
        nc.vector.memset(kct[:, 2:3], float(sc1))
        nc.vector.memset(kct[:, 3:4], float(sc2))
        nc.vector.memset(kct[:, 4:5], float(res_coef))
        nc.vector.memset(kct[:, 5:6], 0.5 * float(nc1v))
        nc.vector.memset(kct[:, 6:7], 0.5 * float(nc2v))
        K_TOTAL = kct[:, 0:1]; K_EPS = kct[:, 1:2]

        psC = ctx.enter_context(tc.tile_pool(name="psC", bufs=4, space="PSUM"))
        psT = ctx.enter_context(tc.tile_pool(name="psT", bufs=2, space="PSUM"))
        psA = ctx.enter_context(tc.tile_pool(name="psA", bufs=1, space="PSUM"))
        psS = ctx.enter_context(tc.tile_pool(name="psS", bufs=1, space="PSUM"))

        work = ctx.enter_context(tc.tile_pool(name="work", bufs=2))
        xt_pool = ctx.enter_context(tc.tile_pool(name="xt", bufs=3))
        io_pool = ctx.enter_context(tc.tile_pool(name="io", bufs=2))

        def bcast(dst_ck, src_1k, k):
            """broadcast [1,k] -> [C,k] via ones matmul."""
            ps = psS.tile([C, 16], F32, tag="small")
            nc.tensor.matmul(out=ps[:, :k], lhsT=ones_row, rhs=src_1k, start=True, stop=True)
            nc.vector.tensor_copy(dst_ck, ps[:, :k])

        x_flat = x_in.rearrange("c h w -> c (h w)")
        cv_flat = cv_d[:].rearrange("c h w -> c (h w)")
        a_flat = a_d[:].rearrange("c h w -> c (h w)")

        def _body(_iv=None):
            def ln_const(ssum, qsum, o_mean, o_rs, ta, tb):
                nc.scalar.mul(out=o_mean, in_=ssum, mul=1.0 / total)
                nc.scalar.mul(out=ta, in_=qsum, mul=1.0 / total)
                nc.vector.tensor_tensor(out=tb, in0=o_mean, in1=o_mean, op=ALU.mult)
                nc.vector.tensor_tensor(out=ta, in0=ta, in1=tb, op=ALU.subtract)
                nc.scalar.activation(out=ta, in_=ta, func=AF.Sqrt, bias=K_EPS, scale=1.0)
                nc.vector.reciprocal(out=o_rs, in_=ta)

            def pe_warm(gate=None, burst=4):
                # junk matmul burst to keep the HAM clock-gate from
                # re-throttling the PE during the stats phases. The first
                # matmul reads `gate` (f32 [C, >=128]) so the burst is paced
                # by the data flow instead of draining immediately.
                wps = psC.tile([C, 512], F32, tag="conv")
                if gate is not None:
                    nc.tensor.matmul(out=wps[:, 0:128], lhsT=identf[:C, :C],
                                     rhs=gate, start=True, stop=True)
                for i in range(burst):
                    nc.tensor.matmul(out=wps[:, 0:512], lhsT=ident[:C, :C],
                                     rhs=jnk_bf, start=(i == 0),
                                     stop=(i == burst - 1))

            _scopes = {}

            def scope(name):
                # close previous scope, open a new one (flat phase markers)
                if _scopes.get("cur"):
                    pn, pid = _scopes["cur"]
                    nc.leave_named_scope(pn, pid, False)
                sid, _ = nc.enter_named_scope(name, False)
                _scopes["cur"] = (name, sid)

            def scope_end():
                if _scopes.get("cur"):
                    pn, pid = _scopes["cur"]
                    nc.leave_named_scope(pn, pid, False)
                    _scopes["cur"] = None

            # ---------------- Phase A pass 1: bn stats ----------------
            scope("A1")
            for ch in range(NCH):
                xt = xt_pool.tile([C, CH], F32, tag="xt")
                nc.sync.dma_start(out=xt, in_=x_flat[:, ch * CH:(ch + 1) * CH])
                if ch == 0:
                    for dst, src in [(aw1_sb, aw1t), (aw2_sb, aw2t),
                                     (aw3_sb, aw3t), (wc2_sb, wc2t),
                                     (wf1_sb, wf1), (wf2_sb, wf2),
                                     (bfull_sb, bfull)]:
                        nc.sync.dma_start(out=dst, in_=src)
                pe_warm(gate=xt[:, 0:128])
                xtv = xt.rearrange("c (a b) -> c a b", a=4)
                for j in range(4):
                    nc.vector.bn_stats(out=bnstats[:, 4 * ch + j, :], in_=xtv[:, j, :])
            nc.vector.bn_aggr(out=mv, in_=bnstats)
            # ex2_i = var + mean^2 ; pm = [mean_i, ex2_i]
            nc.vector.tensor_tensor(out=pm[:, 0:1], in0=mv[:, 0:1], in1=mv[:, 0:1], op=ALU.mult)
            nc.vector.tensor_tensor(out=pm[:, 1:2], in0=mv[:, 1:2], in1=pm[:, 0:1], op=ALU.add)
            nc.vector.tensor_copy(pm[:, 0:1], mv[:, 0:1])
            psm = psS.tile([C, 16], F32, tag="small")
            nc.tensor.matmul(out=psm[:1, 0:2], lhsT=ones_c, rhs=pm, start=True, stop=True)
            Smean = sc[:, 0:1]; Sex2 = sc[:, 1:2]
            nc.vector.tensor_copy(Smean, psm[:1, 0:1])
            nc.vector.tensor_copy(Sex2, psm[:1, 1:2])
            m_ = sc[:, 2:3]
            nc.scalar.mul(out=m_, in_=Smean, mul=1.0 / C)
            Sx2 = sc[:, 3:4]
            nc.scalar.mul(out=Sx2, in_=Sex2, mul=float(HW))
            mm_ = sc[:, 4:5]
            nc.vector.tensor_tensor(out=mm_, in0=m_, in1=m_, op=ALU.mult)
            qtot = sc[:, 5:6]
            nc.vector.scalar_tensor_tensor(out=qtot, in0=mm_, scalar=-total, in1=Sx2,
                                           op0=ALU.mult, op1=ALU.add)
            # broadcast (m, -m)
            nc.vector.tensor_copy(m2[:, 0:1], m_)
            nc.scalar.mul(out=m2[:, 1:2], in_=m_, mul=-1.0)
            bcast(mb[:, 0:2], m2[:, 0:2], 2)
            m96 = mb[:, 0:1]
            negm96 = mb[:, 1:2]

            scope("A2")
            # ---------------- Phase A pass 2: mask/pos/s1/q1 ----------------
            # scalar: rt (accum->s1), rt^2 (accum->q1, odd chunks);
            # vector: mask -> mask_sb (accum->pos), rt^2 (even chunks)
            for ch in range(NCH):
                xt = xt_pool.tile([C, CH], F32, tag="xt")
                nc.sync.dma_start(out=xt, in_=x_flat[:, ch * CH:(ch + 1) * CH])
                pe_warm(gate=xt[:, 0:128])
                rt = work.tile([C, CH], BF16, tag="rt")
                nc.scalar.activation(out=rt, in_=xt, func=AF.Relu, bias=negm96,
                                     scale=1.0, accum_out=s1_part[:, ch:ch + 1])
                junk = work.tile([C, CH], BF16, tag="scr", bufs=1)
                nc.scalar.activation(out=junk, in_=rt, func=AF.Square,
                                     accum_out=q1_part[:, ch:ch + 1])
                nc.vector.tensor_scalar(out=mask_sb[:, ch * CH:(ch + 1) * CH],
                                        in0=rt, scalar1=0.0, scalar2=None,
                                        op0=ALU.is_gt, op1=ALU.add,
                                        accum_out=pos_part[:, ch:ch + 1])
            # reduce partials per partition, then across partitions
            nc.vector.reduce_sum(out=p3[:, 0:1], in_=s1_part, axis=mybir.AxisListType.X)
            nc.vector.reduce_sum(out=p3[:, 1:2], in_=pos_part, axis=mybir.AxisListType.X)
            nc.vector.reduce_sum(out=p3[:, 2:3], in_=q1_part, axis=mybir.AxisListType.X)
            ps3 = psS.tile([C, 16], F32, tag="small")
            nc.tensor.matmul(out=ps3[:1, 0:3], lhsT=ones_c, rhs=p3, start=True, stop=True)
            S1 = sc[:, 6:7]; POS = sc[:, 7:8]; Q1 = sc[:, 9:10]
            nc.vector.tensor_copy(S1, ps3[:1, 0:1])
            nc.vector.tensor_copy(POS, ps3[:1, 1:2])
            nc.vector.tensor_copy(Q1, ps3[:1, 2:3])

            scope("soup")
            # ---------------- scalar soup (simplified) ----------------
            # mean1 == avg1 and mean2 == avg2 exactly, so c1n = c2n = 0:
            #   var1 = (Q1 - avg1^2*POS)/total, scale1 = sqrt(POS/total)/sqrt(var1+eps)
            #   b1 = -scale1*avg1 (= t1a); t2a = -b2
            #   GAM = 0.5*nc1*b1 - 0.5*nc2*b2 - E1*b1 + E2*b2; DEL = 0.5*nc2*b2 - E2*b2
            # All lane math on vector to avoid cross-engine sem latency;
            # one batched sqrt on scalar, batched reciprocals on vector.
            NEG = sc[:, 8:9]  # adjacent to POS for [1,2] batch ops
            PN = sc[:, 7:9]
            nc.vector.tensor_scalar(out=NEG, in0=POS, scalar1=-1.0, scalar2=total,
                                    op0=ALU.mult, op1=ALU.add)
            rPN = sc[:, 10:12]
            nc.vector.reciprocal(out=rPN, in_=PN)
            avg12 = sc[:, 12:14]
            nc.vector.tensor_tensor(out=avg12[:, 0:1], in0=S1, in1=rPN[:, 0:1], op=ALU.mult)
            nc.vector.scalar_tensor_tensor(out=avg12[:, 1:2], in0=S1, scalar=-1.0,
                                           in1=rPN[:, 1:2], op0=ALU.mult, op1=ALU.mult)
            q2 = sc[:, 14:15]
            nc.vector.tensor_tensor(out=q2, in0=qtot, in1=Q1, op=ALU.subtract)
            # nvt12 = avg^2*CNT - Q  (negated var*total)
            t12 = sc[:, 15:17]
            nc.vector.tensor_tensor(out=t12, in0=avg12, in1=PN, op=ALU.mult)
            nvt12 = sc[:, 17:19]
            nc.vector.tensor_tensor(out=nvt12[:, 0:1], in0=t12[:, 0:1], in1=avg12[:, 0:1], op=ALU.mult)
            nc.vector.tensor_tensor(out=nvt12[:, 1:2], in0=t12[:, 1:2], in1=avg12[:, 1:2], op=ALU.mult)
            nc.vector.tensor_tensor(out=nvt12[:, 0:1], in0=nvt12[:, 0:1], in1=Q1, op=ALU.subtract)
            nc.vector.tensor_tensor(out=nvt12[:, 1:2], in0=nvt12[:, 1:2], in1=q2, op=ALU.subtract)
            isq = sc[:, 0:4]  # [var1+eps, var2+eps, POS/total, NEG/total]
            nc.vector.tensor_scalar(out=isq[:, 0:2], in0=nvt12, scalar1=-1.0 / total,
                                    scalar2=eps, op0=ALU.mult, op1=ALU.add)
            nc.vector.tensor_scalar(out=isq[:, 2:4], in0=PN, scalar1=1.0 / total,
                                    scalar2=None, op0=ALU.mult)
            sqa = sc[:, 19:23]
            nc.scalar.activation(out=sqa, in_=isq, func=AF.Sqrt, bias=0.0, scale=1.0)
            rv12 = sc[:, 23:25]
            nc.vector.reciprocal(out=rv12, in_=sqa[:, 0:2])
            scale12 = sc[:, 25:27]
            nc.vector.tensor_tensor(out=scale12, in0=sqa[:, 2:4], in1=rv12, op=ALU.mult)
            # cstv: [a1, b1, a2, t2a=-b2, b2, E1, E2, GAM, DEL]
            a12 = sc[:, 27:29]
            nc.vector.tensor_tensor(out=a12, in0=scale12, in1=kct[:, 2:4], op=ALU.add)
            b12 = sc[:, 29:31]
            nc.vector.scalar_tensor_tensor(out=b12, in0=scale12, scalar=-1.0,
                                           in1=avg12, op0=ALU.mult, op1=ALU.mult)
            p12 = sc[:, 15:17]
            nc.vector.tensor_tensor(out=p12, in0=scale12, in1=kct[:, 5:7], op=ALU.mult)
            nc.vector.tensor_scalar(out=p12, in0=p12, scalar1=float(res_coef),
                                    scalar2=None, op0=ALU.add)
            ra12 = sc[:, 17:19]
            nc.vector.reciprocal(out=ra12, in_=a12)
            E12 = cstv[:, 5:7]
            nc.vector.tensor_tensor(out=E12, in0=p12, in1=ra12, op=ALU.mult)
            qc12 = sc[:, 19:21]  # [q1c, q2c]
            nc.vector.tensor_tensor(out=qc12, in0=b12, in1=kct[:, 5:7], op=ALU.mult)
            eb12 = sc[:, 21:23]  # [E1*b1, E2*b2]
            nc.vector.tensor_tensor(out=eb12, in0=E12, in1=b12, op=ALU.mult)
            GAMv = cstv[:, 7:8]
            nc.vector.tensor_tensor(out=GAMv, in0=qc12[:, 0:1], in1=qc12[:, 1:2], op=ALU.subtract)
            nc.vector.tensor_tensor(out=GAMv, in0=GAMv, in1=eb12[:, 0:1], op=ALU.subtract)
            nc.vector.tensor_tensor(out=GAMv, in0=GAMv, in1=eb12[:, 1:2], op=ALU.add)
            nc.vector.tensor_tensor(out=cstv[:, 8:9], in0=qc12[:, 1:2], in1=eb12[:, 1:2], op=ALU.subtract)
            nc.vector.tensor_copy(cstv[:, 0:1], a12[:, 0:1])
            nc.vector.tensor_copy(cstv[:, 1:2], b12[:, 0:1])
            nc.vector.tensor_copy(cstv[:, 2:3], a12[:, 1:2])
            nc.vector.tensor_scalar(out=cstv[:, 3:4], in0=b12[:, 1:2], scalar1=-1.0,
                                    scalar2=None, op0=ALU.mult)
            nc.vector.tensor_copy(cstv[:, 4:5], b12[:, 1:2])
            bcast(cst[:, 0:9], cstv[:, 0:9], 9)
            E1 = cst[:, 5:6]; E2 = cst[:, 6:7]; GAM = cst[:, 7:8]; DEL = cst[:, 8:9]
            nc.vector.tensor_scalar_mul(out=e1I, in0=ident[:C, :C], scalar1=E1)
            nc.vector.tensor_scalar_mul(out=e2I, in0=ident[:C, :C], scalar1=E2)
            nc.vector.tensor_scalar_mul(out=gI, in0=ident[:C, :C], scalar1=GAM)
            A1 = cst[:, 0:1]; T1A = cst[:, 1:2]
            A2 = cst[:, 2:3]; T2A = cst[:, 3:4]; B2 = cst[:, 4:5]

            # ---------------- Phase A pass 3: emit x1', x2' ----------------
            # scalar: rt, tmp1, tmp2 (from stored mask); vector: nt, x1v, x2v.
            # xr is folded into phase D via diag matmuls (e1I/e2I/gI).
            def emit_chunk(ch):
                xt = xt_pool.tile([C, CH], F32, tag="xt")
                nc.sync.dma_start(out=xt, in_=x_flat[:, ch * CH:(ch + 1) * CH])
                mkv = mask_sb[:, ch * CH:(ch + 1) * CH]
                rt = work.tile([C, CH], BF16, tag="rt")
                nc.scalar.activation(out=rt, in_=xt, func=AF.Relu, bias=negm96, scale=1.0)
                ntb = work.tile([C, CH], BF16, tag="scr", bufs=1)
                nc.vector.tensor_scalar(out=ntb, in0=xt, scalar1=m96, scalar2=0.0,
                                        op0=ALU.subtract, op1=ALU.min)
                tmp1 = work.tile([C, CH], BF16, tag="tmp1")
                nc.scalar.activation(out=tmp1, in_=mkv, func=AF.Identity, bias=0.0,
                                     scale=T1A)
                tmp2 = work.tile([C, CH], BF16, tag="tmp2")
                nc.scalar.activation(out=tmp2, in_=mkv, func=AF.Identity, bias=B2,
                                     scale=T2A)
                x1v = sub_ap(x1pad[:], 1 + ch * 16, 2, 16, W, 1, 1)
                nc.vector.scalar_tensor_tensor(out=x1v, in0=rt, scalar=A1, in1=tmp1,
                                               op0=ALU.mult, op1=ALU.add)
                x2v = sub_ap(x2pad[:], 1 + ch * 16, 2, 16, W, 1, 1)
                nc.vector.scalar_tensor_tensor(out=x2v, in0=ntb, scalar=A2, in1=tmp2,
                                               op0=ALU.mult, op1=ALU.add)

            # wf1 column sums (static) - used later for corr
            psc = psS.tile([C, 16], F32, tag="small")
            nc.tensor.matmul(out=psc[:, 0:1], lhsT=wf1_sb,
                             rhs=ones_c, start=True, stop=True)
            cs1 = st.tile([C, 1], F32)
            nc.vector.tensor_copy(cs1, psc[:, 0:1])

            scope("B")
            # ---------------- Phase B: attention logits (band-major) +
            #                  interleaved cv/value convs ----------------
            def conv_block(yb, w_sb, src_pad, out_cb):
                """One 4-row dense conv block: psum accumulate 9 taps."""
                pt = psC.tile([C, 512], F32, tag="conv")
                for tap in range(KK):
                    dy, dx = divmod(tap, 3)
                    rhs = sub_ap(src_pad[:], yb * 4 + dy, 1 + dx, 4, W, 1, 1)
                    nc.tensor.matmul(out=pt[:, :4 * W], lhsT=w_sb[:, tap, :], rhs=rhs,
                                     start=(tap == 0), stop=(tap == 8))
                out_cb(pt)

            def cv_out(yb):
                def emit(pt):
                    cv_sb = io_pool.tile([C, 4 * W], BF16, tag="cv_sb")
                    nc.scalar.activation(out=cv_sb, in_=pt[:, :4 * W], func=AF.Copy)
                    nc.vector.bn_stats(out=cvstats[:, yb, :], in_=cv_sb)
                    nc.sync.dma_start(out=cv_flat[:, yb * 4 * W:(yb + 1) * 4 * W],
                                      in_=cv_sb)
                return emit

            def val_out(yb):
                def emit(pt):
                    vs = io_pool.tile([C, 4, PW], BF16, tag="vs")
                    nc.vector.memset(vs[:, :, 0:2], 0.0)
                    nc.vector.memset(vs[:, :, W + 2:W + 4], 0.0)
                    nc.scalar.activation(out=vs[:, :, 2:W + 2],
                                         in_=pt[:, :4 * W].rearrange("c (a b) -> c a b", a=4),
                                         func=AF.Copy)
                    nc.sync.dma_start(out=val_d[:, 1 + yb * 4:5 + yb * 4, :], in_=vs)
                return emit

            # Bands of 6 lattice rows (18 image rows). The K/Q convs run
            # DENSE (contiguous rhs, full PE stream rate) into per-band
            # buffers; the stride-3 lattice gather for each kk moves into
            # the transpose's strided input AP.
            LB = 6            # lattice rows per band
            NBAND = Ho // LB  # 7
            # band bi conv needs x rows <= 18*bi+18; chunks are 16 rows.
            chunks_for_band = [(0, 2), (2, 3), (3, 4), (4, 5), (5, 6), (6, 7), (7, 8)]
            conv_for_band = [(0, 7), (7, 11), (11, 15), (15, 19), (19, 23), (23, 27), (27, 32)]
            for bi in range(NBAND):
                for ch in range(*chunks_for_band[bi]):
                    emit_chunk(ch)
                # kdn/qdn layout [C, 3(r), 3(s), LB*42]: lattice pixels of
                # class (r, s) stored contiguously; the psum->SBUF copy does
                # the stride-3 gather with 4D APs.
                kdn = work.tile([C, 3, 3, LB * 42], BF16, tag="kdn", bufs=1)
                qdn = work.tile([C, 3, 3, LB * 42], BF16, tag="qdn", bufs=1)

                def cls_out_ap(dst, blk):
                    b = dst[:, 0, 0, 0]
                    return bass.AP(tensor=b.tensor, offset=b.offset + 42 * blk,
                                   ap=[[dst.ap[0][0], C], [3 * LB * 42 * 3 // 3, 3],
                                       [LB * 42, 3], [1, 42]])

                def cls_in_ap(pt):
                    b = pt[:, 0]
                    return bass.AP(tensor=b.tensor, offset=b.offset,
                                   ap=[[pt.ap[0][0], C], [W, 3], [1, 3], [3, 42]])

                for blk in range(LB):
                    r0 = 18 * bi + 3 * blk
                    for w_sb, src_pad, dst, eng in ((aw1_sb, x1pad, kdn, 0),
                                                    (aw2_sb, x2pad, qdn, 1)):
                        pt = psC.tile([C, 512], F32, tag="conv")
                        for tap in range(KK):
                            dy, dx = divmod(tap, 3)
                            rhs = sub_ap(src_pad[:], r0 + dy, 1 + dx, 3, W, 1, 1)
                            nc.tensor.matmul(out=pt[:, :3 * W], lhsT=w_sb[:, tap, :],
                                             rhs=rhs, start=(tap == 0), stop=(tap == 8))
                        if eng == 0:
                            nc.scalar.activation(out=cls_out_ap(kdn, blk),
                                                 in_=cls_in_ap(pt), func=AF.Copy)
                        else:
                            nc.vector.tensor_copy(cls_out_ap(qdn, blk), cls_in_ap(pt))
                for kk in range(KK):
                    r, s = divmod(kk, 3)
                    attn_ps = psA.tile([C, C], F32, tag="attn")
                    for ci in range(2):
                        in_k = kdn[:, r, s, 126 * ci:126 * ci + 126]
                        in_q = qdn[:, r, s, 126 * ci:126 * ci + 126]
                        ktp = psT.tile([128, C], BF16, tag="tp")
                        qtp = psT.tile([128, C], BF16, tag="tp")
                        nc.tensor.transpose(ktp[:126, :], in_k, ident[:C, :C])
                        nc.tensor.transpose(qtp[:126, :], in_q, ident[:C, :C])
                        kts = work.tile([128, C], BF16, tag="kts")
                        qts = work.tile([128, C], BF16, tag="qts")
                        nc.vector.tensor_copy(kts[:126, :], ktp[:126, :])
                        nc.scalar.activation(out=qts[:126, :], in_=qtp[:126, :], func=AF.Copy)
                        nc.tensor.matmul(out=attn_ps, lhsT=qts[:126, :], rhs=kts[:126, :],
                                         start=(ci == 0), stop=(ci == 1))
                    if bi == 0:
                        nc.vector.tensor_copy(attn_sb[:, kk, :], attn_ps)
                    else:
                        nc.vector.tensor_tensor(out=attn_sb[:, kk, :],
                                                in0=attn_sb[:, kk, :], in1=attn_ps,
                                                op=ALU.add)
                # interleave dense conv blocks whose rows are already emitted
                lo, hi = conv_for_band[bi]
                for yb in range(lo, hi):
                    conv_block(yb, wc2_sb, x2pad, cv_out(yb))
                    conv_block(yb, aw3_sb, x1pad, val_out(yb))

            scope("smax")
            # ---------------- softmax over (kk, c) ----------------
            nc.vector.reduce_max(out=mx, in_=attn_sb, axis=mybir.AxisListType.XY)
            nc.scalar.mul(out=negmx, in_=mx, mul=-1.0)
            nc.scalar.activation(out=attn_sb, in_=attn_sb, func=AF.Exp, bias=negmx,
                                 scale=1.0, accum_out=den)
            nc.vector.reciprocal(out=rden, in_=den)

            # ---------------- Phase B5: w_attn transposes ----------------
            for kk in range(KK):
                tp = psT.tile([128, C], F32, tag="tp")
                nc.tensor.transpose(tp[:C, :], attn_sb[:, kk, :], identf[:C, :C])
                nc.scalar.activation(out=attnT_sb[:, kk, :], in_=tp[:C, :], func=AF.Copy)

            # ---- phase C (cv side, overlaps B6) ----
            nc.vector.bn_aggr(out=cvmv, in_=cvstats)
            nc.vector.tensor_scalar(out=lnp[:, 2:3], in0=cvmv[:, 0:1],
                                    scalar1=float(HW), scalar2=None, op0=ALU.mult)
            nc.vector.tensor_tensor(out=lnp[:, 3:4], in0=cvmv[:, 0:1],
                                    in1=cvmv[:, 0:1], op=ALU.mult)
            nc.vector.tensor_tensor(out=lnp[:, 3:4], in0=lnp[:, 3:4],
                                    in1=cvmv[:, 1:2], op=ALU.add)
            nc.vector.tensor_scalar(out=lnp[:, 3:4], in0=lnp[:, 3:4],
                                    scalar1=float(HW), scalar2=None, op0=ALU.mult)
            pscv = psS.tile([C, 16], F32, tag="small")
            nc.tensor.matmul(out=pscv[:1, 0:2], lhsT=ones_c, rhs=lnp[:, 2:4],
                             start=True, stop=True)
            sCv = lns[:, 2:3]; qCv = lns[:, 3:4]
            nc.vector.tensor_copy(lns[:, 2:4], pscv[:1, 0:2])
            mCv = lns[:, 6:7]; rsCv = lns[:, 7:8]
            tmpa = sc[:, 0:1]; tmpb = sc[:, 1:2]
            ln_const(sCv, qCv, mCv, rsCv, tmpa, tmpb)
            bcast(lnb[:, 0:2], lns[:, 6:8], 2)
            MCV = lnb[:, 0:1]; RSCV = lnb[:, 1:2]
            nc.vector.tensor_scalar_mul(out=wf2s_sb, in0=wf2_sb, scalar1=RSCV)

            scope("B6")
            # ---------------- Phase B6: A conv (stream val stripes) ----------------
            RB = 8  # output rows per block
            nblk = (H + RB - 1) // RB
            for yb in range(nblk):
                rows = min(RB, H - yb * RB)
                vstripe = io_pool.tile([C, RB + 2, PW], BF16, tag="vstripe", bufs=2)
                nc.sync.dma_start(out=vstripe[:, :rows + 2, :],
                                  in_=val_d[:, yb * RB:yb * RB + rows + 2, :])
                for sub in range(rows // 4):
                    i = yb * (RB // 4) + sub
                    pt = psC.tile([C, 512], F32, tag="conv")
                    for tap in range(KK):
                        dy, dx = divmod(tap, 3)
                        rhs = sub_ap(vstripe[:], sub * 4 + dy, 1 + dx, 4, W, 1, 1)
                        nc.tensor.matmul(out=pt[:, :4 * W], lhsT=attnT_sb[:, tap, :],
                                         rhs=rhs, start=(tap == 0), stop=(tap == 8))
                    a_sb = io_pool.tile([C, 4 * W], BF16, tag="a_sb", bufs=3)
                    nc.scalar.activation(out=a_sb, in_=pt[:, :4 * W], func=AF.Copy,
                                         scale=rden)
                    nc.vector.bn_stats(out=astats[:, i, :], in_=a_sb)
                    nc.sync.dma_start(out=a_flat[:, i * 4 * W:(i + 1) * 4 * W], in_=a_sb)

            # ---------------- Phase C: LN consts for A (a side) ----------------
            nc.vector.bn_aggr(out=amv, in_=astats)
            nc.vector.tensor_scalar(out=lnp[:, 0:1], in0=amv[:, 0:1],
                                    scalar1=float(HW), scalar2=None, op0=ALU.mult)
            nc.vector.tensor_tensor(out=lnp[:, 1:2], in0=amv[:, 0:1],
                                    in1=amv[:, 0:1], op=ALU.mult)
            nc.vector.tensor_tensor(out=lnp[:, 1:2], in0=lnp[:, 1:2],
                                    in1=amv[:, 1:2], op=ALU.add)
            nc.vector.tensor_scalar(out=lnp[:, 1:2], in0=lnp[:, 1:2],
                                    scalar1=float(HW), scalar2=None, op0=ALU.mult)
            ps4 = psS.tile([C, 16], F32, tag="small")
            nc.tensor.matmul(out=ps4[:1, 0:2], lhsT=ones_c, rhs=lnp[:, 0:2],
                             start=True, stop=True)
            sA = lns[:, 0:1]; qA = lns[:, 1:2]
            nc.vector.tensor_copy(lns[:, 0:2], ps4[:1, 0:2])
            mA = lns[:, 4:5]; rsA = lns[:, 5:6]
            ln_const(sA, qA, mA, rsA, tmpa, tmpb)
            bcast(lnb[:, 2:4], lns[:, 4:6], 2)
            MA_ = lnb[:, 2:3]; RSA = lnb[:, 3:4]
            nc.vector.tensor_scalar_mul(out=wf1s_sb, in0=wf1_sb, scalar1=RSA)
            # corr = bfull - rsA*mA*colsum(wf1)
            nc.vector.tensor_scalar_mul(out=cs1, in0=cs1, scalar1=RSA)
            nc.vector.tensor_scalar_mul(out=cs1, in0=cs1, scalar1=MA_)
            nc.vector.tensor_tensor(out=corr, in0=bfull_sb, in1=cs1, op=ALU.subtract)
            nc.vector.tensor_tensor(out=delcor, in0=corr, in1=DEL, op=ALU.add)

            scope("D")
            # ---------------- Phase D: final ----------------
            # out = wf1s@a + wf2s@y2t + e1I@x1' + e2I@x2' + gI@mask + delcor
            DW = 8 * W  # two 4-row blocks per iteration
            for yp in range(NT // 2):
                a_in = io_pool.tile([C, DW], BF16, tag="a_in", bufs=3)
                cv_in = io_pool.tile([C, DW], BF16, tag="cv_in", bufs=3)
                nc.sync.dma_start(out=a_in, in_=a_flat[:, yp * DW:(yp + 1) * DW])
                nc.sync.dma_start(out=cv_in, in_=cv_flat[:, yp * DW:(yp + 1) * DW])
                x1w = sub_ap(x1pad[:], 1 + yp * 8, 2, 8, W, 1, 1)
                y2t = work.tile([C, DW], BF16, tag="y2t", bufs=4)
                nc.vector.scalar_tensor_tensor(out=y2t, in0=cv_in, scalar=MCV, in1=x1w,
                                               op0=ALU.subtract, op1=ALU.mult)
                ot = io_pool.tile([C, DW], F32, tag="ot", bufs=2)
                for s in range(2):
                    yb = 2 * yp + s
                    x1v = sub_ap(x1pad[:], 1 + yb * 4, 2, 4, W, 1, 1)
                    x2v = sub_ap(x2pad[:], 1 + yb * 4, 2, 4, W, 1, 1)
                    mkv = mask_sb[:, yb * 4 * W:(yb + 1) * 4 * W]
                    pt = psC.tile([C, 512], F32, tag="conv")
                    nc.tensor.matmul(out=pt[:, :4 * W], lhsT=wf1s_sb,
                                     rhs=a_in[:, s * 4 * W:(s + 1) * 4 * W],
                                     start=True, stop=False)
                    nc.tensor.matmul(out=pt[:, :4 * W], lhsT=wf2s_sb,
                                     rhs=y2t[:, s * 4 * W:(s + 1) * 4 * W],
                                     start=False, stop=False)
                    nc.tensor.matmul(out=pt[:, :4 * W], lhsT=e1I, rhs=x1v,
                                     start=False, stop=False)
                    nc.tensor.matmul(out=pt[:, :4 * W], lhsT=e2I, rhs=x2v,
                                     start=False, stop=False)
                    nc.tensor.matmul(out=pt[:, :4 * W], lhsT=gI, rhs=mkv,
                                     start=False, stop=True)
                    if s == 0:
                        nc.scalar.activation(out=ot[:, s * 4 * W:(s + 1) * 4 * W],
                                             in_=pt[:, :4 * W], func=AF.Identity,
                                             bias=delcor, scale=1.0)
                    else:
                        nc.vector.tensor_scalar(out=ot[:, s * 4 * W:(s + 1) * 4 * W],
                                                in0=pt[:, :4 * W], scalar1=delcor,
                                                scalar2=None, op0=ALU.add)
                nc.scalar.dma_start(
                    out=out_d.rearrange("c h w -> c (h w)")[:, yp * DW:(yp + 1) * DW],
                    in_=ot)

        if repeat == 1:
            _body()
        else:
            with tc.For_i(0, repeat, 1) as _iv:
                _body(_iv)

    _split_multi_waits(nc)
    return nc


_NC_CACHE = {}


def _get_nc(H, W, sc1, sc2, res_coef, nc1v, nc2v):
    key = (H, W, float(sc1), float(sc2), float(res_coef), float(nc1v), float(nc2v))
    if key not in _NC_CACHE:
        nc = bass.Bass("TRN2", target_bir_lowering=False, debug=False)
        build_kernel(nc, H, W, float(sc1), float(sc2), float(res_coef),
                     float(nc1v), float(nc2v))
        _NC_CACHE[key] = nc
    return _NC_CACHE[key]


def _prep_w(w, scale=1.0):
    return np.ascontiguousarray(
        (np.asarray(w, np.float32).transpose(1, 2, 3, 0).reshape(C, 9, C) * scale)
    ).astype(ml_dtypes.bfloat16)


def kernel(x, w_conv2, aw1, aw2, aw3, w_full, b_full, sc1, sc2, res_coef, nc1, nc2):
    x = np.asarray(x, np.float32)
    B, Cc, H, W = x.shape
    assert Cc == C
    nc = _get_nc(H, W, sc1, sc2, res_coef, nc1, nc2)

    inv_s = 1.0 / np.sqrt(C * 9.0)
    w_full = np.asarray(w_full, np.float32)
    shared = {
        "aw1t": _prep_w(aw1, inv_s),
        "aw2t": _prep_w(aw2),
        "aw3t": _prep_w(aw3),
        "wc2t": _prep_w(w_conv2),
        "wf1": np.ascontiguousarray(w_full[:, :C, 0, 0].T).astype(np.float32),
        "wf2": np.ascontiguousarray(w_full[:, C:, 0, 0].T).astype(ml_dtypes.bfloat16),
        "bfull": np.asarray(b_full, np.float32).reshape(C, 1),
    }
    in_maps = [{"x": np.ascontiguousarray(x[b]), **shared} for b in range(B)]
    res = run_bass_kernel_spmd(nc, in_maps, core_ids=list(range(B)))
    return np.stack([res.results[b]["out"] for b in range(B)], axis=0)



# revision 45
# speedup vs baseline: 1.2097x; 1.2097x over previous
"""ButterflyGatingUnit Trainium2 kernel.

Contract: kernel(**inputs) takes the FULL inputs (x: [8, 96, 128, 128] plus
conv weights / scalars) and returns the FULL output [8, 96, 128, 128] f32.
Internally: data-parallel over batch, one example per NeuronCore (8 cores),
via one SPMD Bass program.
"""
import numpy as np
import ml_dtypes
import concourse.bass as bass
import concourse.tile as tile
import concourse.mybir as mybir
from concourse.masks import make_identity
from concourse.bass_utils import run_bass_kernel_spmd
from contextlib import ExitStack

# ---------------------------------------------------------------------------
# Patch TileContext._drain_and_barrier: the walrus build in this toolchain
# rejects Drain instructions carrying more than one sem wait ("Too many sync
# wait commands").  Split the final global-clock wait set across a chain of
# Drain instructions on SP, one wait each.
from concourse.vector_clock import ScopedClock

MAX_WAITS_PER_DRAIN = 1


def _patched_drain_and_barrier(self, tick_clock, wait_clock):
    nc = self.nc
    drain_inst = nc.sync.drain()
    wait_clock.add_sem_waits(
        drain_inst.ins, ScopedClock({None: tick_clock.global_clock})
    )
    inst = drain_inst.ins
    si = inst.sync_info
    waits = list(si.on_wait) if (si and si.on_wait) else []
    if len(waits) > MAX_WAITS_PER_DRAIN:
        si.on_wait = waits[:MAX_WAITS_PER_DRAIN]
        rest = waits[MAX_WAITS_PER_DRAIN:]
        while rest:
            extra = nc.sync.drain()
            extra.ins.sync_info = mybir.SyncInfo(
                on_wait=rest[:MAX_WAITS_PER_DRAIN], on_update=[]
            )
            rest = rest[MAX_WAITS_PER_DRAIN:]

    nc.all_engine_barrier()
    assert self.sems is not None
    popped = nc._tile_sem_poison_stack.pop()
    assert popped is self._sem_poison
    nc.clear_and_free_semaphores(list(self.sems.allocated().values()))
    nc.all_engine_barrier()


tile.TileContext._drain_and_barrier = _patched_drain_and_barrier
# ---------------------------------------------------------------------------

AF = mybir.ActivationFunctionType
ALU = mybir.AluOpType
F32 = mybir.dt.float32
F32R = mybir.dt.float32r
BF16 = mybir.dt.bfloat16
FP8 = mybir.dt.float8e4

C = 96
KK = 9


MAX_WAITS_PER_INST = 1


def _split_multi_waits(nc):
    """This walrus build encodes at most one sem wait per instruction. Hoist
    extra waits onto NoOp carriers inserted just before, on the same engine."""
    f = nc.m.functions[0]
    for blk in f.blocks:
        insts = blk.instructions
        new = []
        changed = False
        ctr = 0
        for inst in insts:
            si = inst.sync_info
            waits = list(si.on_wait) if (si and si.on_wait) else []
            if len(waits) > MAX_WAITS_PER_INST:
                changed = True
                while len(waits) > MAX_WAITS_PER_INST:
                    chunk = waits[:MAX_WAITS_PER_INST]
                    waits = waits[MAX_WAITS_PER_INST:]
                    nop = mybir.InstNoOp(
                        name=f"{inst.name}-wsplit{ctr}", engine=inst.engine,
                        ins=[], outs=[],
                        sync_info=mybir.SyncInfo(on_wait=chunk, on_update=[]))
                    try:
                        nc.register_instruction(nop, overwrite=True)
                    except Exception:
                        pass
                    new.append(nop)
                    ctr += 1
                si.on_wait = waits
            new.append(inst)
        if changed:
            blk.instructions = new


def sub_ap(t_ap, row0, col0, nrow, ncol, rstep, cstep):
    """Strided 3D view [C, nrow, ncol] of a padded [C, PH, PW] SBUF tile."""
    base = t_ap[:, row0, col0]  # AP with offset at (row0, col0), shape [C]
    pstep = t_ap.ap[0][0]
    PW = t_ap.ap[-1][1]  # count of last dim
    row_elems = t_ap.ap[-2][0]  # step of row dim (elements)
    return bass.AP(
        tensor=base.tensor,
        offset=base.offset,
        ap=[[pstep, C], [row_elems * rstep, nrow], [cstep, ncol]],
    )


def build_kernel(nc, H, W, sc1, sc2, res_coef, nc1v, nc2v, repeat=1):
    HW = H * W
    total = float(C * HW)
    Ho = (H - 3) // 3 + 1
    Wo = (W - 3) // 3 + 1
    L = Ho * Wo
    PH, PW = H + 2, W + 4  # interior origin: row 1, col 2 (4B-aligned bf16)
    NT = H // 4            # 4-row blocks
    assert H % 4 == 0 and W % 4 == 0
    eps = 1e-5

    CH = 16 * W            # phase-A chunk: 16 rows
    NCH = H // 16

    # ---------------- DRAM ----------------
    x_in = nc.dram_tensor("x", [C, H, W], F32, kind="ExternalInput").ap()
    aw1t = nc.dram_tensor("aw1t", [C, KK, C], BF16, kind="ExternalInput").ap()
    aw2t = nc.dram_tensor("aw2t", [C, KK, C], BF16, kind="ExternalInput").ap()
    aw3t = nc.dram_tensor("aw3t", [C, KK, C], BF16, kind="ExternalInput").ap()
    wc2t = nc.dram_tensor("wc2t", [C, KK, C], BF16, kind="ExternalInput").ap()
    wf1 = nc.dram_tensor("wf1", [C, C], F32, kind="ExternalInput").ap()
    wf2 = nc.dram_tensor("wf2", [C, C], BF16, kind="ExternalInput").ap()
    bfull = nc.dram_tensor("bfull", [C, 1], F32, kind="ExternalInput").ap()
    out_d = nc.dram_tensor("out", [C, H, W], F32, kind="ExternalOutput").ap()

    with tile.TileContext(nc) as tc, ExitStack() as ctx:
        dram = ctx.enter_context(tc.tile_pool(name="dram", bufs=1, space="DRAM"))
        val_d = dram.tile([C, PH, PW], BF16)
        cv_d = dram.tile([C, H, W], BF16)
        a_d = dram.tile([C, H, W], BF16)

        big = ctx.enter_context(tc.tile_pool(name="big", bufs=1))
        x1pad = big.tile([C, PH, PW], BF16)
        x2pad = big.tile([C, PH, PW], BF16)
        mask_sb = big.tile([C, H * W], FP8)

        wpool = ctx.enter_context(tc.tile_pool(name="wpool", bufs=1))
        aw1_sb = wpool.tile([C, KK, C], BF16)
        aw2_sb = wpool.tile([C, KK, C], BF16)
        aw3_sb = wpool.tile([C, KK, C], BF16)
        wc2_sb = wpool.tile([C, KK, C], BF16)
        wf1_sb = wpool.tile([C, C], F32)
        wf1s_sb = wpool.tile([C, C], BF16)
        wf2_sb = wpool.tile([C, C], BF16)
        wf2s_sb = wpool.tile([C, C], BF16)
        bfull_sb = wpool.tile([C, 1], F32)
        ident = wpool.tile([128, 128], BF16)
        identf = wpool.tile([128, 128], F32)
        ones_c = wpool.tile([C, 1], F32)
        ones_row = wpool.tile([1, C], F32)
        make_identity(nc, ident)
        make_identity(nc, identf)
        nc.vector.memset(ones_c, 1.0)
        nc.vector.memset(ones_row, 1.0)

        # zero pad borders once (interiors are rewritten every iteration)
        for pad in (x1pad, x2pad):
            nc.vector.memset(pad[:, 0, :], 0.0)
            nc.vector.memset(pad[:, PH - 1, :], 0.0)
            nc.vector.memset(pad[:, :, 0:2], 0.0)
            nc.vector.memset(pad[:, :, W + 2:W + 4], 0.0)
        jnk_bf = wpool.tile([C, 512], BF16)
        nc.vector.memset(jnk_bf, 0.0)
        zrow = wpool.tile([C, PW], BF16)
        nc.vector.memset(zrow, 0.0)
        nc.sync.dma_start(out=val_d[:, 0, :], in_=zrow)
        nc.sync.dma_start(out=val_d[:, PH - 1, :], in_=zrow)

        # stats / scalar pools
        st = ctx.enter_context(tc.tile_pool(name="st", bufs=1))
        bnstats = st.tile([C, NT, 6], F32)
        mv = st.tile([C, 2], F32)
        pm = st.tile([C, 2], F32)
        s1_part = st.tile([C, NCH], F32)
        q1_part = st.tile([C, NCH], F32)
        pos_part = st.tile([C, NCH], F32)
        p3 = st.tile([C, 3], F32)
        sc = st.tile([1, 32], F32)      # scalar scratch, [1,1] slices
        cstv = st.tile([1, 16], F32)    # consts to broadcast
        cst = st.tile([C, 16], F32)     # broadcast result
        m2 = st.tile([1, 2], F32)       # (m, -m)
        mb = st.tile([C, 2], F32)       # broadcast (m, -m)
        astats = st.tile([C, NT, 6], F32)
        cvstats = st.tile([C, NT, 6], F32)
        amv = st.tile([C, 2], F32)
        cvmv = st.tile([C, 2], F32)
        lnp = st.tile([C, 4], F32)
        lns = st.tile([1, 8], F32)
        lnb = st.tile([C, 8], F32)
        corr = st.tile([C, 1], F32)
        delcor = st.tile([C, 1], F32)
        e1I = st.tile([C, C], BF16)     # diag(e1) for PE-side xr
        e2I = st.tile([C, C], BF16)
        gI = st.tile([C, C], FP8)
        attn_sb = st.tile([C, KK, C], F32)
        attnT_sb = st.tile([C, KK, C], BF16)
        mx = st.tile([C, 1], F32)
        negmx = st.tile([C, 1], F32)
        den = st.tile([C, 1], F32)
        rden = st.tile([C, 1], F32)
        kct = st.tile([1, 8], F32)   # compile-time consts as [1,1] APs
        nc.vector.memset(kct[:, 0:1], total)
        nc.vector.memset(kct[:, 1:2], eps)
        nc.vector.memset(kct[:, 2:3], float(sc1))
        nc.vector.memset(kct[:, 3:4], float(sc2))
        nc.vector.memset(kct[:, 4:5], float(res_coef))
        nc.vector.memset(kct[:, 5:6], 0.5 * float(nc1v))
        nc.vector.memset(kct[:, 6:7], 0.5 * float(nc2v))
        K_TOTAL = kct[:, 0:1]; K_EPS = kct[:, 1:2]

        psC = ctx.enter_context(tc.tile_pool(name="psC", bufs=4, space="PSUM"))
        psT = ctx.enter_context(tc.tile_pool(name="psT", bufs=2, space="PSUM"))
        psA = ctx.enter_context(tc.tile_pool(name="psA", bufs=1, space="PSUM"))
        psS = ctx.enter_context(tc.tile_pool(name="psS", bufs=1, space="PSUM"))

        work = ctx.enter_context(tc.tile_pool(name="work", bufs=2))
        xt_pool = ctx.enter_context(tc.tile_pool(name="xt", bufs=3))
        io_pool = ctx.enter_context(tc.tile_pool(name="io", bufs=2))

        def bcast(dst_ck, src_1k, k):
            """broadcast [1,k] -> [C,k] via ones matmul."""
            ps = psS.tile([C, 16], F32, tag="small")
            nc.tensor.matmul(out=ps[:, :k], lhsT=ones_row, rhs=src_1k, start=True, stop=True)
            nc.vector.tensor_copy(dst_ck, ps[:, :k])

        x_flat = x_in.rearrange("c h w -> c (h w)")
        cv_flat = cv_d[:].rearrange("c h w -> c (h w)")
        a_flat = a_d[:].rearrange("c h w -> c (h w)")

        def _body(_iv=None):
            def ln_const(ssum, qsum, o_mean, o_rs, ta, tb):
                nc.scalar.mul(out=o_mean, in_=ssum, mul=1.0 / total)
                nc.scalar.mul(out=ta, in_=qsum, mul=1.0 / total)
                nc.vector.tensor_tensor(out=tb, in0=o_mean, in1=o_mean, op=ALU.mult)
                nc.vector.tensor_tensor(out=ta, in0=ta, in1=tb, op=ALU.subtract)
                nc.scalar.activation(out=ta, in_=ta, func=AF.Sqrt, bias=K_EPS, scale=1.0)
                nc.vector.reciprocal(out=o_rs, in_=ta)

            def pe_warm(gate=None, burst=4):
                # junk matmul burst to keep the HAM clock-gate from
                # re-throttling the PE during the stats phases. The first
                # matmul reads `gate` (f32 [C, >=128]) so the burst is paced
                # by the data flow instead of draining immediately.
                wps = psC.tile([C, 512], F32, tag="conv")
                if gate is not None:
                    nc.tensor.matmul(out=wps[:, 0:128], lhsT=identf[:C, :C],
                                     rhs=gate, start=True, stop=True)
                for i in range(burst):
                    nc.tensor.matmul(out=wps[:, 0:512], lhsT=ident[:C, :C],
                                     rhs=jnk_bf, start=(i == 0),
                                     stop=(i == burst - 1))

            _scopes = {}

            def scope(name):
                # close previous scope, open a new one (flat phase markers)
                if _scopes.get("cur"):
                    pn, pid = _scopes["cur"]
                    nc.leave_named_scope(pn, pid, False)
                sid, _ = nc.enter_named_scope(name, False)
                _scopes["cur"] = (name, sid)

            def scope_end():
                if _scopes.get("cur"):
                    pn, pid = _scopes["cur"]
                    nc.leave_named_scope(pn, pid, False)
                    _scopes["cur"] = None

            # ---------------- Phase A pass 1: bn stats ----------------
            scope("A1")
            for ch in range(NCH):
                xt = xt_pool.tile([C, CH], F32, tag="xt")
                h = CH // 2
                nc.sync.dma_start(out=xt[:, :h], in_=x_flat[:, ch * CH:ch * CH + h])
                nc.scalar.dma_start(out=xt[:, h:], in_=x_flat[:, ch * CH + h:(ch + 1) * CH])
                if ch == 0:
                    for dst, src in [(aw1_sb, aw1t), (aw2_sb, aw2t),
                                     (aw3_sb, aw3t), (wc2_sb, wc2t),
                                     (wf1_sb, wf1), (wf2_sb, wf2),
                                     (bfull_sb, bfull)]:
                        nc.sync.dma_start(out=dst, in_=src)
                pe_warm(gate=xt[:, 0:128])
                xtv = xt.rearrange("c (a b) -> c a b", a=4)
                for j in range(4):
                    nc.vector.bn_stats(out=bnstats[:, 4 * ch + j, :], in_=xtv[:, j, :])
            nc.vector.bn_aggr(out=mv, in_=bnstats)
            # ex2_i = var + mean^2 ; pm = [mean_i, ex2_i]
            nc.vector.tensor_tensor(out=pm[:, 0:1], in0=mv[:, 0:1], in1=mv[:, 0:1], op=ALU.mult)
            nc.vector.tensor_tensor(out=pm[:, 1:2], in0=mv[:, 1:2], in1=pm[:, 0:1], op=ALU.add)
            nc.vector.tensor_copy(pm[:, 0:1], mv[:, 0:1])
            psm = psS.tile([C, 16], F32, tag="small")
            nc.tensor.matmul(out=psm[:1, 0:2], lhsT=ones_c, rhs=pm, start=True, stop=True)
            Smean = sc[:, 0:1]; Sex2 = sc[:, 1:2]
            nc.vector.tensor_copy(Smean, psm[:1, 0:1])
            nc.vector.tensor_copy(Sex2, psm[:1, 1:2])
            m_ = sc[:, 2:3]
            nc.scalar.mul(out=m_, in_=Smean, mul=1.0 / C)
            Sx2 = sc[:, 3:4]
            nc.scalar.mul(out=Sx2, in_=Sex2, mul=float(HW))
            mm_ = sc[:, 4:5]
            nc.vector.tensor_tensor(out=mm_, in0=m_, in1=m_, op=ALU.mult)
            qtot = sc[:, 5:6]
            nc.vector.scalar_tensor_tensor(out=qtot, in0=mm_, scalar=-total, in1=Sx2,
                                           op0=ALU.mult, op1=ALU.add)
            # broadcast (m, -m)
            nc.vector.tensor_copy(m2[:, 0:1], m_)
            nc.scalar.mul(out=m2[:, 1:2], in_=m_, mul=-1.0)
            bcast(mb[:, 0:2], m2[:, 0:2], 2)
            m96 = mb[:, 0:1]
            negm96 = mb[:, 1:2]

            scope("A2")
            # ---------------- Phase A pass 2: mask/pos/s1/q1 ----------------
            # scalar: rt (accum->s1), rt^2 (accum->q1, odd chunks);
            # vector: mask -> mask_sb (accum->pos), rt^2 (even chunks)
            for ch in range(NCH):
                xt = xt_pool.tile([C, CH], F32, tag="xt")
                nc.sync.dma_start(out=xt, in_=x_flat[:, ch * CH:(ch + 1) * CH])
                pe_warm(gate=xt[:, 0:128])
                rt = work.tile([C, CH], BF16, tag="rt")
                nc.scalar.activation(out=rt, in_=xt, func=AF.Relu, bias=negm96,
                                     scale=1.0, accum_out=s1_part[:, ch:ch + 1])
                junk = work.tile([C, CH], BF16, tag="scr", bufs=1)
                nc.scalar.activation(out=junk, in_=rt, func=AF.Square,
                                     accum_out=q1_part[:, ch:ch + 1])
                nc.vector.tensor_scalar(out=mask_sb[:, ch * CH:(ch + 1) * CH],
                                        in0=rt, scalar1=0.0, scalar2=None,
                                        op0=ALU.is_gt, op1=ALU.add,
                                        accum_out=pos_part[:, ch:ch + 1])
            # reduce partials per partition, then across partitions
            nc.vector.reduce_sum(out=p3[:, 0:1], in_=s1_part, axis=mybir.AxisListType.X)
            nc.vector.reduce_sum(out=p3[:, 1:2], in_=pos_part, axis=mybir.AxisListType.X)
            nc.vector.reduce_sum(out=p3[:, 2:3], in_=q1_part, axis=mybir.AxisListType.X)
            ps3 = psS.tile([C, 16], F32, tag="small")
            nc.tensor.matmul(out=ps3[:1, 0:3], lhsT=ones_c, rhs=p3, start=True, stop=True)
            S1 = sc[:, 6:7]; POS = sc[:, 7:8]; Q1 = sc[:, 9:10]
            nc.vector.tensor_copy(S1, ps3[:1, 0:1])
            nc.vector.tensor_copy(POS, ps3[:1, 1:2])
            nc.vector.tensor_copy(Q1, ps3[:1, 2:3])

            scope("soup")
            # ---------------- scalar soup (simplified) ----------------
            # mean1 == avg1 and mean2 == avg2 exactly, so c1n = c2n = 0:
            #   var1 = (Q1 - avg1^2*POS)/total, scale1 = sqrt(POS/total)/sqrt(var1+eps)
            #   b1 = -scale1*avg1 (= t1a); t2a = -b2
            #   GAM = 0.5*nc1*b1 - 0.5*nc2*b2 - E1*b1 + E2*b2; DEL = 0.5*nc2*b2 - E2*b2
            # All lane math on vector to avoid cross-engine sem latency;
            # one batched sqrt on scalar, batched reciprocals on vector.
            NEG = sc[:, 8:9]  # adjacent to POS for [1,2] batch ops
            PN = sc[:, 7:9]
            nc.vector.tensor_scalar(out=NEG, in0=POS, scalar1=-1.0, scalar2=total,
                                    op0=ALU.mult, op1=ALU.add)
            rPN = sc[:, 10:12]
            nc.vector.reciprocal(out=rPN, in_=PN)
            avg12 = sc[:, 12:14]
            nc.vector.tensor_tensor(out=avg12[:, 0:1], in0=S1, in1=rPN[:, 0:1], op=ALU.mult)
            nc.vector.scalar_tensor_tensor(out=avg12[:, 1:2], in0=S1, scalar=-1.0,
                                           in1=rPN[:, 1:2], op0=ALU.mult, op1=ALU.mult)
            q2 = sc[:, 14:15]
            nc.vector.tensor_tensor(out=q2, in0=qtot, in1=Q1, op=ALU.subtract)
            # nvt12 = avg^2*CNT - Q  (negated var*total)
            t12 = sc[:, 15:17]
            nc.vector.tensor_tensor(out=t12, in0=avg12, in1=PN, op=ALU.mult)
            nvt12 = sc[:, 17:19]
            nc.vector.tensor_tensor(out=nvt12[:, 0:1], in0=t12[:, 0:1], in1=avg12[:, 0:1], op=ALU.mult)
            nc.vector.tensor_tensor(out=nvt12[:, 1:2], in0=t12[:, 1:2], in1=avg12[:, 1:2], op=ALU.mult)
            nc.vector.tensor_tensor(out=nvt12[:, 0:1], in0=nvt12[:, 0:1], in1=Q1, op=ALU.subtract)
            nc.vector.tensor_tensor(out=nvt12[:, 1:2], in0=nvt12[:, 1:2], in1=q2, op=ALU.subtract)
            isq = sc[:, 0:4]  # [var1+eps, var2+eps, POS/total, NEG/total]
            nc.vector.tensor_scalar(out=isq[:, 0:2], in0=nvt12, scalar1=-1.0 / total,
                                    scalar2=eps, op0=ALU.mult, op1=ALU.add)
            nc.vector.tensor_scalar(out=isq[:, 2:4], in0=PN, scalar1=1.0 / total,
                                    scalar2=None, op0=ALU.mult)
            sqa = sc[:, 19:23]
            nc.scalar.activation(out=sqa, in_=isq, func=AF.Sqrt, bias=0.0, scale=1.0)
            rv12 = sc[:, 23:25]
            nc.vector.reciprocal(out=rv12, in_=sqa[:, 0:2])
            scale12 = sc[:, 25:27]
            nc.vector.tensor_tensor(out=scale12, in0=sqa[:, 2:4], in1=rv12, op=ALU.mult)
            # cstv: [a1, b1, a2, t2a=-b2, b2, E1, E2, GAM, DEL]
            a12 = sc[:, 27:29]
            nc.vector.tensor_tensor(out=a12, in0=scale12, in1=kct[:, 2:4], op=ALU.add)
            b12 = sc[:, 29:31]
            nc.vector.scalar_tensor_tensor(out=b12, in0=scale12, scalar=-1.0,
                                           in1=avg12, op0=ALU.mult, op1=ALU.mult)
            p12 = sc[:, 15:17]
            nc.vector.tensor_tensor(out=p12, in0=scale12, in1=kct[:, 5:7], op=ALU.mult)
            nc.vector.tensor_scalar(out=p12, in0=p12, scalar1=float(res_coef),
                                    scalar2=None, op0=ALU.add)
            nc.vector.tensor_copy(cstv[:, 0:1], a12[:, 0:1])
            nc.vector.tensor_copy(cstv[:, 1:2], b12[:, 0:1])
            nc.vector.tensor_copy(cstv[:, 2:3], a12[:, 1:2])
            nc.vector.tensor_scalar(out=cstv[:, 3:4], in0=b12[:, 1:2], scalar1=-1.0,
                                    scalar2=None, op0=ALU.mult)
            nc.vector.tensor_copy(cstv[:, 4:5], b12[:, 1:2])
            bcast(cst[:, 0:5], cstv[:, 0:5], 5)
            ra12 = sc[:, 17:19]
            nc.vector.reciprocal(out=ra12, in_=a12)
            E12 = cstv[:, 5:7]
            nc.vector.tensor_tensor(out=E12, in0=p12, in1=ra12, op=ALU.mult)
            qc12 = sc[:, 19:21]  # [q1c, q2c]
            nc.vector.tensor_tensor(out=qc12, in0=b12, in1=kct[:, 5:7], op=ALU.mult)
            eb12 = sc[:, 21:23]  # [E1*b1, E2*b2]
            nc.vector.tensor_tensor(out=eb12, in0=E12, in1=b12, op=ALU.mult)
            GAMv = cstv[:, 7:8]
            nc.vector.tensor_tensor(out=GAMv, in0=qc12[:, 0:1], in1=qc12[:, 1:2], op=ALU.subtract)
            nc.vector.tensor_tensor(out=GAMv, in0=GAMv, in1=eb12[:, 0:1], op=ALU.subtract)
            nc.vector.tensor_tensor(out=GAMv, in0=GAMv, in1=eb12[:, 1:2], op=ALU.add)
            nc.vector.tensor_tensor(out=cstv[:, 8:9], in0=qc12[:, 1:2], in1=eb12[:, 1:2], op=ALU.subtract)
            bcast(cst[:, 5:9], cstv[:, 5:9], 4)
            E1 = cst[:, 5:6]; E2 = cst[:, 6:7]; GAM = cst[:, 7:8]; DEL = cst[:, 8:9]
            nc.vector.tensor_scalar_mul(out=e1I, in0=ident[:C, :C], scalar1=E1)
            nc.vector.tensor_scalar_mul(out=e2I, in0=ident[:C, :C], scalar1=E2)
            nc.vector.tensor_scalar_mul(out=gI, in0=ident[:C, :C], scalar1=GAM)
            A1 = cst[:, 0:1]; T1A = cst[:, 1:2]
            A2 = cst[:, 2:3]; T2A = cst[:, 3:4]; B2 = cst[:, 4:5]

            # ---------------- Phase A pass 3: emit x1', x2' ----------------
            # scalar: rt, tmp1, tmp2 (from stored mask); vector: nt, x1v, x2v.
            # xr is folded into phase D via diag matmuls (e1I/e2I/gI).
            def emit_chunk(ch):
                xt = xt_pool.tile([C, CH], F32, tag="xt")
                nc.sync.dma_start(out=xt, in_=x_flat[:, ch * CH:(ch + 1) * CH])
                mkv = mask_sb[:, ch * CH:(ch + 1) * CH]
                rt = work.tile([C, CH], BF16, tag="rt")
                nc.scalar.activation(out=rt, in_=xt, func=AF.Relu, bias=negm96, scale=1.0)
                ntb = work.tile([C, CH], BF16, tag="scr", bufs=1)
                nc.vector.tensor_scalar(out=ntb, in0=xt, scalar1=m96, scalar2=0.0,
                                        op0=ALU.subtract, op1=ALU.min)
                tmp1 = work.tile([C, CH], BF16, tag="tmp1")
                nc.scalar.activation(out=tmp1, in_=mkv, func=AF.Identity, bias=0.0,
                                     scale=T1A)
                tmp2 = work.tile([C, CH], BF16, tag="tmp2")
                nc.scalar.activation(out=tmp2, in_=mkv, func=AF.Identity, bias=B2,
                                     scale=T2A)
                x1v = sub_ap(x1pad[:], 1 + ch * 16, 2, 16, W, 1, 1)
                nc.vector.scalar_tensor_tensor(out=x1v, in0=rt, scalar=A1, in1=tmp1,
                                               op0=ALU.mult, op1=ALU.add)
                x2v = sub_ap(x2pad[:], 1 + ch * 16, 2, 16, W, 1, 1)
                nc.vector.scalar_tensor_tensor(out=x2v, in0=ntb, scalar=A2, in1=tmp2,
                                               op0=ALU.mult, op1=ALU.add)

            # wf1 column sums (static) - used later for corr
            psc = psS.tile([C, 16], F32, tag="small")
            nc.tensor.matmul(out=psc[:, 0:1], lhsT=wf1_sb,
                             rhs=ones_c, start=True, stop=True)
            cs1 = st.tile([C, 1], F32)
            nc.vector.tensor_copy(cs1, psc[:, 0:1])

            scope("B")
            # ---------------- Phase B: attention logits (band-major) +
            #                  interleaved cv/value convs ----------------
            def conv_block(yb, w_sb, src_pad, out_cb):
                """One 4-row dense conv block: psum accumulate 9 taps."""
                pt = psC.tile([C, 512], F32, tag="conv")
                for tap in range(KK):
                    dy, dx = divmod(tap, 3)
                    rhs = sub_ap(src_pad[:], yb * 4 + dy, 1 + dx, 4, W, 1, 1)
                    nc.tensor.matmul(out=pt[:, :4 * W], lhsT=w_sb[:, tap, :], rhs=rhs,
                                     start=(tap == 0), stop=(tap == 8))
                out_cb(pt)

            def cv_out(yb):
                def emit(pt):
                    cv_sb = io_pool.tile([C, 4 * W], BF16, tag="cv_sb")
                    nc.scalar.activation(out=cv_sb, in_=pt[:, :4 * W], func=AF.Copy)
                    nc.vector.bn_stats(out=cvstats[:, yb, :], in_=cv_sb)
                    nc.sync.dma_start(out=cv_flat[:, yb * 4 * W:(yb + 1) * 4 * W],
                                      in_=cv_sb)
                return emit

            def val_out(yb):
                def emit(pt):
                    vs = io_pool.tile([C, 4, PW], BF16, tag="vs")
                    nc.vector.memset(vs[:, :, 0:2], 0.0)
                    nc.vector.memset(vs[:, :, W + 2:W + 4], 0.0)
                    nc.scalar.activation(out=vs[:, :, 2:W + 2],
                                         in_=pt[:, :4 * W].rearrange("c (a b) -> c a b", a=4),
                                         func=AF.Copy)
                    nc.sync.dma_start(out=val_d[:, 1 + yb * 4:5 + yb * 4, :], in_=vs)
                return emit

            # Bands of 6 lattice rows (18 image rows). The K/Q convs run
            # DENSE (contiguous rhs, full PE stream rate) into per-band
            # buffers; the stride-3 lattice gather for each kk moves into
            # the transpose's strided input AP.
            LB = 6            # lattice rows per band
            NBAND = Ho // LB  # 7
            chunks_for_band = [(0, 2), (2, 3), (3, 4), (4, 5), (5, 6), (6, 7), (7, 8)]
            conv_for_band = [(0, 7), (7, 11), (11, 15), (15, 19), (19, 23), (23, 27), (27, 32)]
            for bi in range(NBAND):
                lr0 = 6 * bi
                for ch in range(*chunks_for_band[bi]):
                    emit_chunk(ch)
                # kdn/qdn layout [C, 3(r), 3(s), LB*42]: lattice pixels of
                # class (r, s) stored contiguously; the psum->SBUF copy does
                # the stride-3 gather with 4D APs.
                kdn = work.tile([C, 3, 3, 6 * 42], BF16, tag="kdn", bufs=1)
                qdn = work.tile([C, 3, 3, 6 * 42], BF16, tag="qdn", bufs=1)

                def cls_out_ap(dst, blk):
                    b = dst[:, 0, 0, 0]
                    return bass.AP(tensor=b.tensor, offset=b.offset + 42 * blk,
                                   ap=[[dst.ap[0][0], C], [3 * 6 * 42, 3],
                                       [6 * 42, 3], [1, 42]])

                def cls_in_ap(pt):
                    b = pt[:, 0]
                    return bass.AP(tensor=b.tensor, offset=b.offset,
                                   ap=[[pt.ap[0][0], C], [W, 3], [1, 3], [3, 42]])

                for blk in range(LB):
                    r0 = 3 * (lr0 + blk)
                    for w_sb, src_pad, dst, eng in ((aw1_sb, x1pad, kdn, 0),
                                                    (aw2_sb, x2pad, qdn, 1)):
                        pt = psC.tile([C, 512], F32, tag="conv")
                        for tap in range(KK):
                            dy, dx = divmod(tap, 3)
                            rhs = sub_ap(src_pad[:], r0 + dy, 1 + dx, 3, W, 1, 1)
                            nc.tensor.matmul(out=pt[:, :3 * W], lhsT=w_sb[:, tap, :],
                                             rhs=rhs, start=(tap == 0), stop=(tap == 8))
                        if eng == 0:
                            nc.scalar.activation(out=cls_out_ap(kdn, blk),
                                                 in_=cls_in_ap(pt), func=AF.Copy)
                        else:
                            nc.vector.tensor_copy(cls_out_ap(qdn, blk), cls_in_ap(pt))
                nci = LB // 3
                for kk in range(KK):
                    r, s = divmod(kk, 3)
                    attn_ps = psA.tile([C, C], F32, tag="attn")
                    for ci in range(nci):
                        in_k = kdn[:, r, s, 126 * ci:126 * ci + 126]
                        in_q = qdn[:, r, s, 126 * ci:126 * ci + 126]
                        ktp = psT.tile([128, C], BF16, tag="tp")
                        qtp = psT.tile([128, C], BF16, tag="tp")
                        nc.tensor.transpose(ktp[:126, :], in_k, ident[:C, :C])
                        nc.tensor.transpose(qtp[:126, :], in_q, ident[:C, :C])
                        kts = work.tile([128, C], BF16, tag="kts")
                        qts = work.tile([128, C], BF16, tag="qts")
                        nc.vector.tensor_copy(kts[:126, :], ktp[:126, :])
                        nc.scalar.activation(out=qts[:126, :], in_=qtp[:126, :], func=AF.Copy)
                        nc.tensor.matmul(out=attn_ps, lhsT=qts[:126, :], rhs=kts[:126, :],
                                         start=(ci == 0), stop=(ci == nci - 1))
                    if bi == 0:
                        nc.vector.tensor_copy(attn_sb[:, kk, :], attn_ps)
                    else:
                        nc.vector.tensor_tensor(out=attn_sb[:, kk, :],
                                                in0=attn_sb[:, kk, :], in1=attn_ps,
                                                op=ALU.add)
                # interleave dense conv blocks whose rows are already emitted
                lo, hi = conv_for_band[bi]
                for yb in range(lo, hi):
                    conv_block(yb, wc2_sb, x2pad, cv_out(yb))
                    conv_block(yb, aw3_sb, x1pad, val_out(yb))

            scope("smax")
            # ---------------- softmax over (kk, c) ----------------
            nc.vector.reduce_max(out=mx, in_=attn_sb, axis=mybir.AxisListType.XY)
            nc.scalar.mul(out=negmx, in_=mx, mul=-1.0)
            nc.scalar.activation(out=attn_sb, in_=attn_sb, func=AF.Exp, bias=negmx,
                                 scale=1.0, accum_out=den)
            nc.vector.reciprocal(out=rden, in_=den)

            # ---------------- Phase B5: w_attn transposes ----------------
            for kk in range(KK):
                tp = psT.tile([128, C], F32, tag="tp")
                nc.tensor.transpose(tp[:C, :], attn_sb[:, kk, :], identf[:C, :C])
                nc.scalar.activation(out=attnT_sb[:, kk, :], in_=tp[:C, :], func=AF.Copy)

            # ---- phase C (cv side, overlaps B6) ----
            nc.vector.bn_aggr(out=cvmv, in_=cvstats)
            nc.vector.tensor_scalar(out=lnp[:, 2:3], in0=cvmv[:, 0:1],
                                    scalar1=float(HW), scalar2=None, op0=ALU.mult)
            nc.vector.tensor_tensor(out=lnp[:, 3:4], in0=cvmv[:, 0:1],
                                    in1=cvmv[:, 0:1], op=ALU.mult)
            nc.vector.tensor_tensor(out=lnp[:, 3:4], in0=lnp[:, 3:4],
                                    in1=cvmv[:, 1:2], op=ALU.add)
            nc.vector.tensor_scalar(out=lnp[:, 3:4], in0=lnp[:, 3:4],
                                    scalar1=float(HW), scalar2=None, op0=ALU.mult)
            pscv = psS.tile([C, 16], F32, tag="small")
            nc.tensor.matmul(out=pscv[:1, 0:2], lhsT=ones_c, rhs=lnp[:, 2:4],
                             start=True, stop=True)
            sCv = lns[:, 2:3]; qCv = lns[:, 3:4]
            nc.vector.tensor_copy(lns[:, 2:4], pscv[:1, 0:2])
            mCv = lns[:, 6:7]; rsCv = lns[:, 7:8]
            tmpa = sc[:, 0:1]; tmpb = sc[:, 1:2]
            ln_const(sCv, qCv, mCv, rsCv, tmpa, tmpb)
            bcast(lnb[:, 0:2], lns[:, 6:8], 2)
            MCV = lnb[:, 0:1]; RSCV = lnb[:, 1:2]
            nc.vector.tensor_scalar_mul(out=wf2s_sb, in0=wf2_sb, scalar1=RSCV)

            scope("B6")
            # ---------------- Phase B6: A conv (stream val stripes) ----------------
            RB = 8  # output rows per block
            nblk = (H + RB - 1) // RB
            for yb in range(nblk):
                rows = min(RB, H - yb * RB)
                vstripe = io_pool.tile([C, RB + 2, PW], BF16, tag="vstripe", bufs=2)
                nc.sync.dma_start(out=vstripe[:, :rows + 2, :],
                                  in_=val_d[:, yb * RB:yb * RB + rows + 2, :])
                for sub in range(rows // 4):
                    i = yb * (RB // 4) + sub
                    pt = psC.tile([C, 512], F32, tag="conv")
                    for tap in range(KK):
                        dy, dx = divmod(tap, 3)
                        rhs = sub_ap(vstripe[:], sub * 4 + dy, 1 + dx, 4, W, 1, 1)
                        nc.tensor.matmul(out=pt[:, :4 * W], lhsT=attnT_sb[:, tap, :],
                                         rhs=rhs, start=(tap == 0), stop=(tap == 8))
                    a_sb = io_pool.tile([C, 4 * W], BF16, tag="a_sb", bufs=3)
                    nc.scalar.activation(out=a_sb, in_=pt[:, :4 * W], func=AF.Copy,
                                         scale=rden)
                    nc.vector.bn_stats(out=astats[:, i, :], in_=a_sb)
                    nc.sync.dma_start(out=a_flat[:, i * 4 * W:(i + 1) * 4 * W], in_=a_sb)

            # ---------------- Phase C: LN consts for A (a side) ----------------
            nc.vector.bn_aggr(out=amv, in_=astats)
            nc.vector.tensor_scalar(out=lnp[:, 0:1], in0=amv[:, 0:1],
                                    scalar1=float(HW), scalar2=None, op0=ALU.mult)
            nc.vector.tensor_tensor(out=lnp[:, 1:2], in0=amv[:, 0:1],
                                    in1=amv[:, 0:1], op=ALU.mult)
            nc.vector.tensor_tensor(out=lnp[:, 1:2], in0=lnp[:, 1:2],
                                    in1=amv[:, 1:2], op=ALU.add)
            nc.vector.tensor_scalar(out=lnp[:, 1:2], in0=lnp[:, 1:2],
                                    scalar1=float(HW), scalar2=None, op0=ALU.mult)
            ps4 = psS.tile([C, 16], F32, tag="small")
            nc.tensor.matmul(out=ps4[:1, 0:2], lhsT=ones_c, rhs=lnp[:, 0:2],
                             start=True, stop=True)
            sA = lns[:, 0:1]; qA = lns[:, 1:2]
            nc.vector.tensor_copy(lns[:, 0:2], ps4[:1, 0:2])
            mA = lns[:, 4:5]; rsA = lns[:, 5:6]
            ln_const(sA, qA, mA, rsA, tmpa, tmpb)
            bcast(lnb[:, 2:4], lns[:, 4:6], 2)
            MA_ = lnb[:, 2:3]; RSA = lnb[:, 3:4]
            nc.vector.tensor_scalar_mul(out=wf1s_sb, in0=wf1_sb, scalar1=RSA)
            # corr = bfull - rsA*mA*colsum(wf1)
            nc.vector.tensor_scalar_mul(out=cs1, in0=cs1, scalar1=RSA)
            nc.vector.tensor_scalar_mul(out=cs1, in0=cs1, scalar1=MA_)
            nc.vector.tensor_tensor(out=corr, in0=bfull_sb, in1=cs1, op=ALU.subtract)
            nc.vector.tensor_tensor(out=delcor, in0=corr, in1=DEL, op=ALU.add)

            scope("D")
            # ---------------- Phase D: final ----------------
            # out = wf1s@a + wf2s@y2t + e1I@x1' + e2I@x2' + gI@mask + delcor
            DW = 8 * W  # two 4-row blocks per iteration
            for yp in range(NT // 2):
                a_in = io_pool.tile([C, DW], BF16, tag="a_in", bufs=4)
                cv_in = io_pool.tile([C, DW], BF16, tag="cv_in", bufs=4)
                nc.sync.dma_start(out=a_in, in_=a_flat[:, yp * DW:(yp + 1) * DW])
                nc.sync.dma_start(out=cv_in, in_=cv_flat[:, yp * DW:(yp + 1) * DW])
                x1w = sub_ap(x1pad[:], 1 + yp * 8, 2, 8, W, 1, 1)
                y2t = work.tile([C, DW], BF16, tag="y2t", bufs=4)
                nc.vector.scalar_tensor_tensor(out=y2t, in0=cv_in, scalar=MCV, in1=x1w,
                                               op0=ALU.subtract, op1=ALU.mult)
                ot = io_pool.tile([C, DW], F32, tag="ot", bufs=2)
                for s in range(2):
                    yb = 2 * yp + s
                    x1v = sub_ap(x1pad[:], 1 + yb * 4, 2, 4, W, 1, 1)
                    x2v = sub_ap(x2pad[:], 1 + yb * 4, 2, 4, W, 1, 1)
                    mkv = mask_sb[:, yb * 4 * W:(yb + 1) * 4 * W]
                    pt = psC.tile([C, 512], F32, tag="conv")
                    nc.tensor.matmul(out=pt[:, :4 * W], lhsT=wf1s_sb,
                                     rhs=a_in[:, s * 4 * W:(s + 1) * 4 * W],
                                     start=True, stop=False)
                    nc.tensor.matmul(out=pt[:, :4 * W], lhsT=wf2s_sb,
                                     rhs=y2t[:, s * 4 * W:(s + 1) * 4 * W],
                                     start=False, stop=False)
                    nc.tensor.matmul(out=pt[:, :4 * W], lhsT=e1I, rhs=x1v,
                                     start=False, stop=False)
                    nc.tensor.matmul(out=pt[:, :4 * W], lhsT=e2I, rhs=x2v,
                                     start=False, stop=False)
                    nc.tensor.matmul(out=pt[:, :4 * W], lhsT=gI, rhs=mkv,
                                     start=False, stop=True)
                    nc.scalar.activation(out=ot[:, s * 4 * W:(s + 1) * 4 * W],
                                         in_=pt[:, :4 * W], func=AF.Identity,
                                         bias=delcor, scale=1.0)
                nc.scalar.dma_start(
                    out=out_d.rearrange("c h w -> c (h w)")[:, yp * DW:(yp + 1) * DW],
                    in_=ot)

        if repeat == 1:
            _body()
        else:
            with tc.For_i(0, repeat, 1) as _iv:
                _body(_iv)

    _split_multi_waits(nc)
    return nc


_NC_CACHE = {}


def _get_nc(H, W, sc1, sc2, res_coef, nc1v, nc2v):
    key = (H, W, float(sc1), float(sc2), float(res_coef), float(nc1v), float(nc2v))
    if key not in _NC_CACHE:
        nc = bass.Bass("TRN2", target_bir_lowering=False, debug=False)
        build_kernel(nc, H, W, float(sc1), float(sc2), float(res_coef),
                     float(nc1v), float(nc2v))
        _NC_CACHE[key] = nc
    return _NC_CACHE[key]


def _prep_w(w, scale=1.0):
    return np.ascontiguousarray(
        (np.asarray(w, np.float32).transpose(1, 2, 3, 0).reshape(C, 9, C) * scale)
    ).astype(ml_dtypes.bfloat16)


def kernel(x, w_conv2, aw1, aw2, aw3, w_full, b_full, sc1, sc2, res_coef, nc1, nc2):
    x = np.asarray(x, np.float32)
    B, Cc, H, W = x.shape
    assert Cc == C
    nc = _get_nc(H, W, sc1, sc2, res_coef, nc1, nc2)

    inv_s = 1.0 / np.sqrt(C * 9.0)
    w_full = np.asarray(w_full, np.float32)
    shared = {
        "aw1t": _prep_w(aw1, inv_s),
        "aw2t": _prep_w(aw2),
        "aw3t": _prep_w(aw3),
        "wc2t": _prep_w(w_conv2),
        "wf1": np.ascontiguousarray(w_full[:, :C, 0, 0].T).astype(np.float32),
        "wf2": np.ascontiguousarray(w_full[:, C:, 0, 0].T).astype(ml_dtypes.bfloat16),
        "bfull": np.asarray(b_full, np.float32).reshape(C, 1),
    }
    in_maps = [{"x": np.ascontiguousarray(x[b]), **shared} for b in range(B)]
    res = run_bass_kernel_spmd(nc, in_maps, core_ids=list(range(B)))
    return np.stack([res.results[b]["out"] for b in range(B)], axis=0)

